# revision 2
# baseline (speedup 1.0000x reference)
"""Eval-mode ClassConditionalBatchNorm2d on 8 Trainium2 NeuronCores.

Math: for each sample b with label l:
    use_class = (alpha > 0) & (class_counts[l] >= 100)
    mean/var  = blend of (global, class[l]) stats if use_class else global
    out       = (x - mean) / sqrt(var + eps) * weight + bias

This folds to a per-(sample, channel) affine:  out = x * scale + shift with
    scale[b,c] = weight[c] / sqrt(var[b,c] + eps)
    shift[b,c] = bias[c] - mean[b,c] * scale[b,c]

The [B=64, C=256] scale/shift tables are tiny (64 KB) and computed on host;
the device kernel streams x through SBUF applying one fused DVE
tensor_scalar (mult+add, per-partition scalars) per (sample, channel-half) —
pure HBM streaming, bound by the ~358 GB/s per-core HBM limit.

Precision: the accuracy budget (rel err 2e-2 vs |out|max) is ~40x looser
than fp16 rounding (~2^-11 per value, ~1e-3 scale-relative for the
in+out pair), so x is staged to device HBM as fp16 and the kernel writes
fp16 output (host upcasts to f32).  That halves HBM traffic vs f32:
2 x 12.85 MB per core -> ~71.8 us roofline instead of ~143.5 us.

Sharding: pure data parallel over batch. Each of the 8 cores gets 8 samples
(x shard [8, 256, 56*56]) plus its own [128, 32] f32 scale/shift table
arranged so that column 4*b + 2*h + {0,1} holds (scale, shift) for sample b,
channel half h, with channels on partitions. Tiles cover `fuse` whole
samples so each load/store is a single large DMA fanning across all 16 SDMA
ports; input/output tile pools pipeline load/compute/store.
"""

import numpy as np
from contextlib import ExitStack

B, C, H, W = 64, 256, 56, 56
HW = H * W
N_CORES = 8
BPC = B // N_CORES  # samples per core
N_HALF = C // 128   # channel halves (partition tiles)
EPS = 1e-5
MIN_COUNT = 100.0

# Device-side dtypes ("float32" | "float16" | "bfloat16").
IN_DT = "float16"
OUT_DT = "float16"
TAB_DT = "float32"

# Pipeline shape (tuned on HW; see bench.py).
DEFAULT = dict(bufs=4, obufs=3, fuse=1, in_place=False, store_eng="sync")

_PROGRAM_CACHE = {}
LAST_RESULTS = None  # BassKernelResults of the most recent run


def _np_dt(name):
    if name == "bfloat16":
        import ml_dtypes

        return np.dtype(ml_dtypes.bfloat16)
    return np.dtype(name)


def _build_program(iters=1, dyn_loop=None, bufs=4, obufs=3, fuse=1,
                   in_place=False, store_eng="sync",
                   in_dt=None, out_dt=None, tab_dt=None, split=1):
    """Build + compile the single-core SPMD Bass program (cached).

    iters > 1 repeats the identical sweep back-to-back inside one NEFF;
    dyn_loop=N wraps the sweep in a hardware For loop of N trips (bench use).
    fuse=G loads/stores G whole samples (both channel halves) per DMA.
    split=S cuts each tile DMA into S free-dim chunks (same tile, S DMAs).
    in_place applies the affine into the input tile (requires in_dt==out_dt).
    store_eng: which engine issues store DMAs ("sync"|"scalar"|"gpsimd").
    """
    in_dt = IN_DT if in_dt is None else in_dt
    out_dt = OUT_DT if out_dt is None else out_dt
    tab_dt = TAB_DT if tab_dt is None else tab_dt
    key = (iters, dyn_loop, bufs, obufs, fuse, in_place, store_eng,
           in_dt, out_dt, tab_dt, split)
    if key in _PROGRAM_CACHE:
        return _PROGRAM_CACHE[key]

    import concourse.tile as tile
    from concourse import bacc, mybir

    i_dt = getattr(mybir.dt, in_dt)
    o_dt = getattr(mybir.dt, out_dt)
    t_dt = getattr(mybir.dt, tab_dt)
    if in_place:
        assert in_dt == out_dt, "in_place needs matching dtypes"

    nc = bacc.Bacc(
        "TRN2", target_bir_lowering=False, debug=False, num_devices=N_CORES
    )
    x_ap = nc.dram_tensor("x", [BPC, C, HW], i_dt, kind="ExternalInput").ap()
    tab_ap = nc.dram_tensor(
        "tables", [128, BPC * N_HALF * 2], t_dt, kind="ExternalInput"
    ).ap()
    out_ap = nc.dram_tensor("out", [BPC, C, HW], o_dt, kind="ExternalOutput").ap()

    with tile.TileContext(nc) as tc:
        with ExitStack() as ctx:
            tabp = ctx.enter_context(tc.tile_pool(name="tab", bufs=1))
            xp = ctx.enter_context(tc.tile_pool(name="xs", bufs=bufs))
            outp = None
            if not in_place:
                outp = ctx.enter_context(tc.tile_pool(name="os", bufs=obufs))
            st_eng = getattr(nc, store_eng)

            tab = tabp.tile([128, BPC * N_HALF * 2], t_dt)
            nc.sync.dma_start(tab[:], tab_ap[:])

            fw = HW // split

            def sweep():
                G = fuse  # samples per tile
                for b0 in range(0, BPC, G):
                    t = xp.tile([128, G * N_HALF, HW], i_dt)
                    src = x_ap[b0 : b0 + G].rearrange(
                        "g (h p) f -> p (g h) f", h=N_HALF
                    )
                    for s in range(split):
                        nc.sync.dma_start(
                            t[:, :, s * fw : (s + 1) * fw],
                            src[:, :, s * fw : (s + 1) * fw],
                        )
                    o = t if in_place else outp.tile([128, G * N_HALF, HW], o_dt)
                    for j in range(G * N_HALF):
                        r = N_HALF * b0 + j
                        nc.vector.tensor_scalar(
                            o[:, j, :],
                            t[:, j, :],
                            tab[:, 2 * r : 2 * r + 1],
                            tab[:, 2 * r + 1 : 2 * r + 2],
                            mybir.AluOpType.mult,
                            mybir.AluOpType.add,
                        )
                    dst = out_ap[b0 : b0 + G].rearrange(
                        "g (h p) f -> p (g h) f", h=N_HALF
                    )
                    for s in range(split):
                        st_eng.dma_start(
                            dst[:, :, s * fw : (s + 1) * fw],
                            o[:, :, s * fw : (s + 1) * fw],
                        )

            if dyn_loop is not None:
                with tc.For_i(0, dyn_loop, 1):
                    for _ in range(iters):
                        sweep()
            else:
                for _ in range(iters):
                    sweep()

    nc.compile()
    _PROGRAM_CACHE[key] = nc
    return nc


def _scale_shift(labels, weight, bias, global_mean, global_var,
                 class_mean, class_var, class_counts, alpha):
    """Per-sample affine tables [B, C], mirroring the reference's f32 branch
    selection exactly; the weight/sqrt fold is done in f64 for accuracy."""
    labels = np.asarray(labels).astype(np.int64).reshape(-1)
    a = np.float32(np.asarray(alpha).reshape(()))
    one_m_a = np.float32(1.0) - a

    use_class = (float(a) > 0.0) & (
        np.asarray(class_counts, np.float32)[labels] >= np.float32(MIN_COUNT)
    )  # [B]
    gm = np.asarray(global_mean, np.float32)
    gv = np.asarray(global_var, np.float32)
    blend_mean = one_m_a * gm[None, :] + a * np.asarray(class_mean, np.float32)[labels]
    blend_var = np.clip(
        one_m_a * gv[None, :] + a * np.asarray(class_var, np.float32)[labels],
        np.float32(EPS),
        None,
    )
    mean = np.where(use_class[:, None], blend_mean, gm[None, :])  # [B, C] f32
    var = np.where(use_class[:, None], blend_var, gv[None, :])

    scale64 = np.asarray(weight, np.float64)[None, :] / np.sqrt(
        var.astype(np.float64) + np.float64(EPS)
    )
    shift64 = np.asarray(bias, np.float64)[None, :] - mean.astype(np.float64) * scale64
    return scale64.astype(np.float32), shift64.astype(np.float32)


def make_in_maps(inputs):
    """Shard + stage the full inputs: per-core x shard (cast to IN_DT) and
    the per-core [128, BPC*N_HALF*2] scale/shift table (col = 4b + 2h + k)."""
    x = np.asarray(inputs["x"])
    scale, shift = _scale_shift(
        inputs["labels"], inputs["weight"], inputs["bias"],
        inputs["global_mean"], inputs["global_var"],
        inputs["class_mean"], inputs["class_var"],
        inputs["class_counts"], inputs["alpha"],
    )
    idt = _np_dt(IN_DT)
    tdt = _np_dt(TAB_DT)
    in_maps = []
    for c in range(N_CORES):
        xs = x[c * BPC : (c + 1) * BPC].reshape(BPC, C, HW).astype(idt)
        sc = scale[c * BPC : (c + 1) * BPC].reshape(BPC, N_HALF, 128)
        sh = shift[c * BPC : (c + 1) * BPC].reshape(BPC, N_HALF, 128)
        st = np.stack([sc, sh], axis=-1)  # [b, h, p, 2]
        tab = np.ascontiguousarray(
            st.transpose(2, 0, 1, 3).reshape(128, BPC * N_HALF * 2)
        ).astype(tdt)
        in_maps.append({"x": np.ascontiguousarray(xs), "tables": tab})
    return in_maps


def gather_output(res):
    out = np.empty((B, C, H, W), np.float32)
    for c in range(N_CORES):
        out[c * BPC : (c + 1) * BPC] = np.asarray(
            res.results[c]["out"], dtype=np.float32
        ).reshape(BPC, C, H, W)
    return out


def kernel(x, labels, weight, bias, global_mean, global_var,
           class_mean, class_var, class_counts, alpha):
    global LAST_RESULTS
    from concourse.bass_utils import run_bass_kernel_spmd

    in_maps = make_in_maps(dict(
        x=x, labels=labels, weight=weight, bias=bias,
        global_mean=global_mean, global_var=global_var,
        class_mean=class_mean, class_var=class_var,
        class_counts=class_counts, alpha=alpha,
    ))
    nc = _build_program(**DEFAULT)
    res = run_bass_kernel_spmd(nc, in_maps, list(range(N_CORES)))
    LAST_RESULTS = res
    return gather_output(res)


# revision 22
# speedup vs baseline: 2.1358x; 2.1358x over previous
"""Eval-mode ClassConditionalBatchNorm2d on 8 Trainium2 NeuronCores.

Math: for each sample b with label l:
    use_class = (alpha > 0) & (class_counts[l] >= 100)
    mean/var  = blend of (global, class[l]) stats if use_class else global
    out       = (x - mean) / sqrt(var + eps) * weight + bias

This folds to a per-(sample, channel) affine:  out = x * scale + shift with
    scale[b,c] = weight[c] / sqrt(var[b,c] + eps)
    shift[b,c] = bias[c] - mean[b,c] * scale[b,c]

The [B=64, C=256] scale/shift tables are tiny (64 KB) and computed on host;
the device kernel streams x through SBUF applying one fused DVE
tensor_scalar (mult+add, per-partition scalars) per (sample, channel-half) —
pure HBM streaming, bound by the ~358 GB/s per-core HBM limit.

Precision: the accuracy budget (rel err 2e-2 vs |out|max) is ~40x looser
than fp16 rounding (~2^-11 per value, ~1e-3 scale-relative for the
in+out pair), so x is staged to device HBM as fp16 and the kernel writes
fp16 output (host upcasts to f32).  That halves HBM traffic vs f32:
2 x 12.85 MB per core -> ~71.8 us roofline instead of ~143.5 us.

Sharding: pure data parallel over batch. Each of the 8 cores gets 8 samples
(x shard [8, 256, 56*56]) plus its own [128, 32] f32 scale/shift table
arranged so that column 4*b + 2*h + {0,1} holds (scale, shift) for sample b,
channel half h, with channels on partitions. Tiles cover `fuse` whole
samples so each load/store is a single large DMA fanning across all 16 SDMA
ports; input/output tile pools pipeline load/compute/store.
"""

import numpy as np
from contextlib import ExitStack

B, C, H, W = 64, 256, 56, 56
HW = H * W
N_CORES = 8
BPC = B // N_CORES  # samples per core
N_HALF = C // 128   # channel halves (partition tiles)
EPS = 1e-5
MIN_COUNT = 100.0

# Device-side dtypes ("float32" | "float16" | "bfloat16").
IN_DT = "float16"
OUT_DT = "float16"
TAB_DT = "float32"
# "c": x/out staged [BPC, C, HW]; "p": partition-major [128, BPC*N_HALF, HW]
LAYOUT = "c"

# Pipeline shape (tuned on HW; see bench.py).  fuse=2 -> 3.2 MB DMAs
# (4 loads + 4 stores per sweep) measured best across sessions: ~79-80 us
# vs 82-84 us for fuse=1, 82+ for fuse=4, with tail-splitting, alternate
# HWDGE rings, deeper buffering, and in-place variants all neutral-to-worse.
DEFAULT = dict(bufs=4, obufs=3, fuse=2, in_place=False, store_eng="sync")

_PROGRAM_CACHE = {}
LAST_RESULTS = None  # BassKernelResults of the most recent run


def _np_dt(name):
    if name == "bfloat16":
        import ml_dtypes

        return np.dtype(ml_dtypes.bfloat16)
    return np.dtype(name)


def _build_program(iters=1, dyn_loop=None, bufs=4, obufs=3, fuse=1,
                   in_place=False, store_eng="sync",
                   in_dt=None, out_dt=None, tab_dt=None, split=1,
                   tail_split=1, layout=None, variant="full"):
    """Build + compile the single-core SPMD Bass program (cached).

    iters > 1 repeats the identical sweep back-to-back inside one NEFF;
    dyn_loop=N wraps the sweep in a hardware For loop of N trips (bench use).
    fuse=G loads/stores G whole samples (both channel halves) per DMA.
    split=S cuts each tile DMA into S free-dim chunks (same tile, S DMAs).
    in_place applies the affine into the input tile (requires in_dt==out_dt).
    store_eng: which engine issues store DMAs ("sync"|"scalar"|"gpsimd").
    """
    in_dt = IN_DT if in_dt is None else in_dt
    out_dt = OUT_DT if out_dt is None else out_dt
    tab_dt = TAB_DT if tab_dt is None else tab_dt
    layout = LAYOUT if layout is None else layout
    key = (iters, dyn_loop, bufs, obufs, fuse, in_place, store_eng,
           in_dt, out_dt, tab_dt, split, tail_split, layout, variant)
    if key in _PROGRAM_CACHE:
        return _PROGRAM_CACHE[key]

    import concourse.tile as tile
    from concourse import bacc, mybir

    i_dt = getattr(mybir.dt, in_dt)
    o_dt = getattr(mybir.dt, out_dt)
    t_dt = getattr(mybir.dt, tab_dt)
    if in_place:
        assert in_dt == out_dt, "in_place needs matching dtypes"

    nc = bacc.Bacc(
        "TRN2", target_bir_lowering=False, debug=False, num_devices=N_CORES
    )
    if layout == "p":
        # Partition-major staging: host pre-transposes so each partition's
        # data is one contiguous run per DMA group (max descriptor size).
        x_ap = nc.dram_tensor(
            "x", [128, BPC * N_HALF, HW], i_dt, kind="ExternalInput"
        ).ap()
        out_ap = nc.dram_tensor(
            "out", [128, BPC * N_HALF, HW], o_dt, kind="ExternalOutput"
        ).ap()
    else:
        x_ap = nc.dram_tensor("x", [BPC, C, HW], i_dt, kind="ExternalInput").ap()
        out_ap = nc.dram_tensor("out", [BPC, C, HW], o_dt, kind="ExternalOutput").ap()
    tab_ap = nc.dram_tensor(
        "tables", [128, BPC * N_HALF * 2], t_dt, kind="ExternalInput"
    ).ap()

    with tile.TileContext(nc) as tc:
        with ExitStack() as ctx:
            tabp = ctx.enter_context(tc.tile_pool(name="tab", bufs=1))
            xp = ctx.enter_context(tc.tile_pool(name="xs", bufs=bufs))
            outp = None
            if not in_place:
                outp = ctx.enter_context(tc.tile_pool(name="os", bufs=obufs))
            if store_eng == "alt":
                # Alternate both loads and stores across the two HWDGE rings
                # (sync / scalar), opposite phases for load vs store.
                engs = [nc.sync, nc.scalar]
                ld_of = lambda i: engs[i % 2]
                st_of = lambda i: engs[(i + 1) % 2]
            else:
                ld_of = lambda i: nc.sync
                st_of = lambda i: getattr(nc, store_eng)

            tab = tabp.tile([128, BPC * N_HALF * 2], t_dt)
            nc.sync.dma_start(tab[:], tab_ap[:])

            src_tile = None
            if variant == "dve":
                srcp = ctx.enter_context(tc.tile_pool(name="src", bufs=1))
                src_tile = srcp.tile([128, fuse * N_HALF, HW], i_dt)
                nc.vector.memset(src_tile[:], 1.0)

            fw = HW // split

            def sweep():
                G = fuse  # samples per tile
                for b0 in range(0, BPC, G):
                    t = src_tile if variant == "dve" else xp.tile(
                        [128, G * N_HALF, HW], i_dt
                    )
                    if layout == "p":
                        src = x_ap[:, b0 * N_HALF : (b0 + G) * N_HALF, :]
                    else:
                        src = x_ap[b0 : b0 + G].rearrange(
                            "g (h p) f -> p (g h) f", h=N_HALF
                        )
                    if variant != "dve":
                        for s in range(split):
                            ld_of(b0 // G).dma_start(
                                t[:, :, s * fw : (s + 1) * fw],
                                src[:, :, s * fw : (s + 1) * fw],
                            )
                    o = t if (in_place or variant == "dma") else outp.tile(
                        [128, G * N_HALF, HW], o_dt
                    )
                    if variant != "dma":
                        for j in range(G * N_HALF):
                            r = N_HALF * b0 + j
                            nc.vector.tensor_scalar(
                                o[:, j, :],
                                t[:, j, :],
                                tab[:, 2 * r : 2 * r + 1],
                                tab[:, 2 * r + 1 : 2 * r + 2],
                                mybir.AluOpType.mult,
                                mybir.AluOpType.add,
                            )
                    if layout == "p":
                        dst = out_ap[:, b0 * N_HALF : (b0 + G) * N_HALF, :]
                    else:
                        dst = out_ap[b0 : b0 + G].rearrange(
                            "g (h p) f -> p (g h) f", h=N_HALF
                        )
                    if variant != "dve":
                        # Split the LAST group's store into small chunks so
                        # the unoverlapped drain tail is short.
                        last = b0 + G >= BPC
                        ts = tail_split * split if last else split
                        tfw = HW // ts
                        for s in range(ts):
                            st_of(b0 // G).dma_start(
                                dst[:, :, s * tfw : (s + 1) * tfw],
                                o[:, :, s * tfw : (s + 1) * tfw],
                            )

            if dyn_loop is not None:
                with tc.For_i(0, dyn_loop, 1):
                    for _ in range(iters):
                        sweep()
            else:
                for _ in range(iters):
                    sweep()

    nc.compile()
    _PROGRAM_CACHE[key] = nc
    return nc


def _scale_shift(labels, weight, bias, global_mean, global_var,
                 class_mean, class_var, class_counts, alpha):
    """Per-sample affine tables [B, C], mirroring the reference's f32 branch
    selection exactly; the weight/sqrt fold is done in f64 for accuracy."""
    labels = np.asarray(labels).astype(np.int64).reshape(-1)
    a = np.float32(np.asarray(alpha).reshape(()))
    one_m_a = np.float32(1.0) - a

    use_class = (float(a) > 0.0) & (
        np.asarray(class_counts, np.float32)[labels] >= np.float32(MIN_COUNT)
    )  # [B]
    gm = np.asarray(global_mean, np.float32)
    gv = np.asarray(global_var, np.float32)
    blend_mean = one_m_a * gm[None, :] + a * np.asarray(class_mean, np.float32)[labels]
    blend_var = np.clip(
        one_m_a * gv[None, :] + a * np.asarray(class_var, np.float32)[labels],
        np.float32(EPS),
        None,
    )
    mean = np.where(use_class[:, None], blend_mean, gm[None, :])  # [B, C] f32
    var = np.where(use_class[:, None], blend_var, gv[None, :])

    scale64 = np.asarray(weight, np.float64)[None, :] / np.sqrt(
        var.astype(np.float64) + np.float64(EPS)
    )
    shift64 = np.asarray(bias, np.float64)[None, :] - mean.astype(np.float64) * scale64
    return scale64.astype(np.float32), shift64.astype(np.float32)


def make_in_maps(inputs):
    """Shard + stage the full inputs: per-core x shard (cast to IN_DT) and
    the per-core [128, BPC*N_HALF*2] scale/shift table (col = 4b + 2h + k)."""
    x = np.asarray(inputs["x"])
    scale, shift = _scale_shift(
        inputs["labels"], inputs["weight"], inputs["bias"],
        inputs["global_mean"], inputs["global_var"],
        inputs["class_mean"], inputs["class_var"],
        inputs["class_counts"], inputs["alpha"],
    )
    idt = _np_dt(IN_DT)
    tdt = _np_dt(TAB_DT)
    in_maps = []
    for c in range(N_CORES):
        xs = x[c * BPC : (c + 1) * BPC].reshape(BPC, C, HW).astype(idt)
        if LAYOUT == "p":
            xs = np.ascontiguousarray(
                xs.reshape(BPC, N_HALF, 128, HW).transpose(2, 0, 1, 3)
            ).reshape(128, BPC * N_HALF, HW)
        sc = scale[c * BPC : (c + 1) * BPC].reshape(BPC, N_HALF, 128)
        sh = shift[c * BPC : (c + 1) * BPC].reshape(BPC, N_HALF, 128)
        st = np.stack([sc, sh], axis=-1)  # [b, h, p, 2]
        tab = np.ascontiguousarray(
            st.transpose(2, 0, 1, 3).reshape(128, BPC * N_HALF * 2)
        ).astype(tdt)
        in_maps.append({"x": np.ascontiguousarray(xs), "tables": tab})
    return in_maps


def gather_output(res):
    out = np.empty((B, C, H, W), np.float32)
    for c in range(N_CORES):
        o = np.asarray(res.results[c]["out"], dtype=np.float32)
        if LAYOUT == "p":
            o = o.reshape(128, BPC, N_HALF, HW).transpose(1, 2, 0, 3)
        out[c * BPC : (c + 1) * BPC] = o.reshape(BPC, C, H, W)
    return out


def kernel(x, labels, weight, bias, global_mean, global_var,
           class_mean, class_var, class_counts, alpha):
    global LAST_RESULTS
    from concourse.bass_utils import run_bass_kernel_spmd

    in_maps = make_in_maps(dict(
        x=x, labels=labels, weight=weight, bias=bias,
        global_mean=global_mean, global_var=global_var,
        class_mean=class_mean, class_var=class_var,
        class_counts=class_counts, alpha=alpha,
    ))
    nc = _build_program(**DEFAULT)
    res = run_bass_kernel_spmd(nc, in_maps, list(range(N_CORES)))
    LAST_RESULTS = res
    return gather_output(res)
